# revision 10
# baseline (speedup 1.0000x reference)
"""ClassCapsule dynamic-routing kernel for 8 Trainium2 NeuronCores.

Problem (hardcoded shapes):
    x:    [64, 2048, 16]  fp32
    W:    [2048, 16, 1024] fp32
    bias: [64, 16]        fp32
    out:  [64, 64, 16]    fp32  (squeezed v after 3 routing iterations)

Strategy (in_caps-sharded, s-AllReduce per iteration):
  - IC=2048 in_caps split across 8 cores (256 each); every core holds all
    64 batches.  W traffic per core is 1/8th of the replicated layout.
  - u_hat tiles are [128=(s8,b16), 4096=(d16,g4,n64)] bf16 per 8-in_cap
    block: the (d, g, n) column order makes every d-tree-reduce level a
    contiguous slice.
  - Projection runs in two passes: pass 1 streams W and accumulates the
    iteration-0 s (c0 uniform) as a dense GEMM over K=(in_cap,e), so its
    AllReduce is posted early and hides under pass 2 (the u_hat matmuls
    with a full-K block-diagonal x operand + PSUM->SBUF copies + spill).
  - u_hat kept SBUF-resident for the first RESB block groups, spilled to
    DRAM per block group [128,4096] and re-read in iters 1-2 (prefetched
    across the AllReduce gap; spill-out uses the vector-engine DMA queue
    so it never serializes with the sync-queue loads).
  - Per iteration per block: agreement product + contiguous 4-level tree
    reduce (DVE), softmax over n (ACT exp, DVE smalls), c replicated over
    d by a broadcast copy on the scalar engine, cu product (DVE), s
    accumulated with selector matmuls (PE), then an 8-core fp32 AllReduce
    of the partial s [64,1024] and squash.  GPSIMD is intentionally idle:
    its SBUF port pair is an exclusive lock shared with the DVE, so any
    GPSIMD tensor work just serializes against the DVE.
"""

import numpy as np
import ml_dtypes

import concourse.bass as bass
import concourse.tile as tile
from concourse import bacc, mybir
from concourse.bass_utils import run_bass_kernel_spmd

# ---------------------------------------------------------------- constants
B, IC, E = 64, 2048, 16          # batch, in_caps, in_dim
NCAP, D = 64, 16                 # n_caps, cap_dim
ND = NCAP * D                    # 1024
CORES = 8
ICL = IC // CORES                # 256 local in_caps
NBLK = ICL // 8                  # 32 blocks of 8 in_caps
NT = NBLK * 4                    # 128 u_hat tiles [(s,b16), 1024]
RESB = 9                        # block groups resident in SBUF
GND = 4 * ND                     # group width (4096)
EPS = 1e-7

FP = mybir.dt.float32
BF = mybir.dt.bfloat16
BF_NP = ml_dtypes.bfloat16


def _host_prep(x, W, bias):
    """Build per-core host-side tensors (all bf16 except bias)."""
    wxbd_all = []
    xbd_all = []
    for c in range(CORES):
        i0 = c * ICL
        w8 = W[i0:i0 + ICL].reshape(NBLK, 128, ND)          # [(blk),(s,e),nd]
        # reorder W columns (n,d) -> (d,n)
        w8 = w8.reshape(NBLK, 128, NCAP, D).transpose(0, 1, 3, 2)
        w8 = np.ascontiguousarray(w8).reshape(NBLK, 128, ND)
        xc = x[:, i0:i0 + ICL, :]                           # [64, 256, 16]
        # xr[blk, s, e, bg, b] = x[bg*16+b, i0+blk*8+s, e]
        xr = xc.transpose(1, 2, 0).reshape(NBLK, 8, E, 4, 16)
        wxbd_all.append(w8.astype(BF_NP))
        # block-diagonal x: xbd[blk, bg, s*16+e, s*16+b] = xr[blk, s, e, bg, b]
        xbd = np.zeros((NBLK, 4, 128, 128), np.float32)
        for s in range(8):
            xbd[:, :, s * 16:(s + 1) * 16, s * 16:(s + 1) * 16] = \
                xr[:, s].transpose(0, 2, 1, 3)
        xbd_all.append(xbd.astype(BF_NP))

    # selector: sel[p, g*64 + m] = 1 if m == g*16 + p%16
    sel = np.zeros((128, 4 * NCAP), np.float32)
    p = np.arange(128)
    for g in range(4):
        sel[p, g * NCAP + g * 16 + (p % 16)] = 1.0
    sel = sel.astype(BF_NP)

    # bias in (d, n) order
    bias_dn = np.ascontiguousarray(bias.T.reshape(1, ND)).astype(np.float32)
    return wxbd_all, xbd_all, sel, bias_dn


def _build_program():
    nc = bacc.Bacc("TRN2", target_bir_lowering=False, num_devices=CORES)

    wxbd_d = nc.dram_tensor("wxbd", [NBLK, 128, ND], BF,
                            kind="ExternalInput")
    xbd_d = nc.dram_tensor("xbd", [NBLK, 4, 128, 128], BF,
                           kind="ExternalInput")
    sel_d = nc.dram_tensor("sel", [128, 4 * NCAP], BF, kind="ExternalInput")
    bias_d = nc.dram_tensor("bias_flat", [1, ND], FP, kind="ExternalInput")
    v_out = nc.dram_tensor("v_out", [B, ND], FP, kind="ExternalOutput")

    u_spill_d = nc.dram_tensor("u_spill", [NBLK - RESB, 128, GND], BF)
    v_scr = [nc.dram_tensor(f"v_scr{r}", [B, ND], BF) for r in range(2)]
    ar_in = [nc.dram_tensor(f"ar_in{r}", [B, ND], FP) for r in range(3)]
    ar_out = [nc.dram_tensor(f"ar_out{r}", [B, ND], FP, addr_space="Shared")
              for r in range(3)]

    with tile.TileContext(nc) as tc:
        with (
            tc.tile_pool(name="wp", bufs=4) as wp,
            tc.tile_pool(name="xp", bufs=4) as xp,         # xbd stream
            tc.tile_pool(name="ul", bufs=3) as ulp,        # spill load/stage
            tc.tile_pool(name="tp", bufs=1) as tp,         # u*v products
            tc.tile_pool(name="crp", bufs=2) as crp,       # c replicated
            tc.tile_pool(name="cup", bufs=2) as cup,       # u*c products
            tc.tile_pool(name="smalls", bufs=2) as sp,
            tc.tile_pool(name="sq", bufs=1) as qp,         # squash [64,*]
            tc.tile_pool(name="consts", bufs=1) as cp,
            tc.tile_pool(name="vbp", bufs=1) as vbp,
            tc.tile_pool(name="ures", bufs=1) as urp,
            tc.tile_pool(name="bstate", bufs=1) as bsp,
            tc.tile_pool(name="ps", bufs=3, space="PSUM") as psp,
            tc.tile_pool(name="ps_acc", bufs=1, space="PSUM") as psa,
        ):
            # ---- constants resident in SBUF
            sel_sb = cp.tile([128, 4 * NCAP], BF)
            nc.sync.dma_start(out=sel_sb, in_=sel_d[:, :])
            bias_sb = cp.tile([B, ND], FP)
            bias_src = bass.AP(tensor=bias_d, offset=0, ap=[[0, B], [1, ND]])
            nc.sync.dma_start(out=bias_sb, in_=bias_src)
            eps_t = cp.tile([B, 1], FP)
            nc.vector.memset(eps_t, EPS)

            # persistent state
            u_res = urp.tile([128, RESB * GND], BF)        # resident u_hat
            b_all = bsp.tile([128, NT * NCAP], BF)         # routing logits

            # ---------------- squash helper: v = squash(s*scale + bias)
            def squash(s_in, scale, bf16=False):
                s_sb = qp.tile([B, ND], FP, tag="s_sb")
                nc.vector.scalar_tensor_tensor(
                    out=s_sb, in0=s_in, scalar=float(scale), in1=bias_sb,
                    op0=mybir.AluOpType.mult, op1=mybir.AluOpType.add)
                sq = qp.tile([B, ND], FP, tag="sq")
                nc.vector.tensor_mul(sq, s_sb, s_sb)
                # contiguous tree reduce over d (columns are (d, n))
                nc.vector.tensor_add(sq[:, 0:512], sq[:, 0:512],
                                     sq[:, 512:1024])
                nc.vector.tensor_add(sq[:, 0:256], sq[:, 0:256], sq[:, 256:512])
                nc.vector.tensor_add(sq[:, 0:128], sq[:, 0:128], sq[:, 128:256])
                nsq = sp.tile([B, NCAP], FP, tag="nsq")
                nc.vector.tensor_add(nsq, sq[:, 0:64], sq[:, 64:128])
                norm = sp.tile([B, NCAP], FP, tag="norm")
                nc.scalar.activation(out=norm, in_=nsq,
                                     func=mybir.ActivationFunctionType.Sqrt,
                                     bias=eps_t[:, :], scale=1.0)
                den = sp.tile([B, NCAP], FP, tag="den")
                nc.vector.scalar_tensor_tensor(
                    out=den, in0=nsq, scalar=float(EPS + 1.0), in1=norm,
                    op0=mybir.AluOpType.add, op1=mybir.AluOpType.mult)
                rden = sp.tile([B, NCAP], FP, tag="rden")
                nc.vector.reciprocal(out=rden, in_=den)
                fac = sp.tile([B, NCAP], FP, tag="fac")
                nc.vector.scalar_tensor_tensor(
                    out=fac, in0=nsq, scalar=float(EPS), in1=rden,
                    op0=mybir.AluOpType.add, op1=mybir.AluOpType.mult)
                if bf16:
                    out_t = qp.tile([B, ND], BF, tag="v_bf16", name="v_bf16")
                else:
                    out_t = qp.tile([B, ND], FP, tag="v_sb", name="v_sb")
                fac_b = bass.AP(tensor=fac.tensor, offset=fac.offset,
                                ap=[list(fac.ap[0]), [0, D], [1, NCAP]])
                with nc.allow_low_precision(reason="bf16 v for broadcast"):
                    nc.vector.tensor_mul(
                        out_t.rearrange("p (d n) -> p d n", n=NCAP),
                        s_sb.rearrange("p (d n) -> p d n", n=NCAP),
                        fac_b)
                return out_t

            def allreduce_post(s_ps, r):
                """partial s (PSUM): copy out and launch the AllReduce."""
                s_loc = qp.tile([B, ND], FP, tag="s_loc")
                nc.scalar.copy(out=s_loc, in_=s_ps)
                nc.scalar.dma_start(out=ar_in[r][:, :], in_=s_loc)
                nc.gpsimd.collective_compute(
                    "AllReduce",
                    mybir.AluOpType.add,
                    replica_groups=[list(range(CORES))],
                    ins=[ar_in[r][:, :].opt()],
                    outs=[ar_out[r][:, :].opt()],
                )

            def allreduce_fetch(r):
                s_glob = qp.tile([B, ND], FP, tag="s_glob")
                nc.scalar.dma_start(out=s_glob, in_=ar_out[r][:, :])
                return s_glob

            def broadcast_v(v_bf, r):
                """v [64,(d,n)] bf16 -> vb [128, (d,g,n)] bf16."""
                nc.scalar.dma_start(out=v_scr[r][:, :], in_=v_bf)
                vb = vbp.tile([128, GND], BF, tag="vb", name="vb")
                for g in range(4):
                    # dest cols (d, n) for fixed g; src v_scr[b(g,b16), (d,n)]
                    dst = bass.AP(tensor=vb.tensor,
                                  offset=vb.offset + g * NCAP,
                                  ap=[list(vb.ap[0]), [4 * NCAP, D], [1, NCAP]])
                    src = bass.AP(tensor=v_scr[r], offset=g * 16 * ND,
                                  ap=[[0, 8], [ND, 16], [NCAP, D], [1, NCAP]])
                    nc.scalar.dma_start(out=dst, in_=src)
                return vb

            # spill prefetch bookkeeping
            pref = {}

            def prefetch_spills(n):
                for blk in range(RESB, min(RESB + n, NBLK)):
                    t = ulp.tile([128, GND], BF, tag="u4", name="u4")
                    nc.sync.dma_start(out=t, in_=u_spill_d[blk - RESB])
                    pref[blk] = t

            # ========== Phase P: u_hat projection + iter-0 s + spill =======
            s0_ps = psa.tile([B, ND], FP, tag="s_acc")
            for blk in range(NBLK):
                wt = wp.tile([128, ND], BF, tag="w")
                nc.sync.dma_start(out=wt, in_=wxbd_d[blk])
                xt = xp.tile([128, 512], BF, tag="x")
                xsrc = bass.AP(tensor=xbd_d, offset=blk * 4 * 128 * 128,
                               ap=[[128, 128], [128 * 128, 4], [1, 128]])
                nc.sync.dma_start(out=xt, in_=xsrc)
                if blk < RESB:
                    u4 = u_res[:, blk * GND:(blk + 1) * GND]
                else:
                    u4 = ulp.tile([128, GND], BF, tag="u4")
                for g in range(4):
                    u_ps = psp.tile([128, ND], FP, tag="u_ps")
                    for nh in range(2):
                        nc.tensor.matmul(
                            u_ps[:, nh * 512:(nh + 1) * 512],
                            xt[:, g * 128:(g + 1) * 128],
                            wt[:, nh * 512:(nh + 1) * 512],
                            start=True, stop=True,
                            skip_group_check=True)
                    # copy u_ps [128,(d,n)] into u4 cols (d,g,n) at slice g
                    u_dst = bass.AP(tensor=u4.tensor,
                                    offset=u4.offset + g * NCAP,
                                    ap=[list(u4.ap[0]), [4 * NCAP, D],
                                        [1, NCAP]])
                    u_src = u_ps.rearrange("p (d n) -> p d n", n=NCAP)
                    if g % 2 == 0:
                        nc.scalar.copy(out=u_dst, in_=u_src)
                    else:
                        nc.vector.tensor_copy(out=u_dst, in_=u_src)
                    # iter-0 s: selector matmul on the finished u4 slice
                    t = blk * 4 + g
                    for nh in range(2):
                        rhs = bass.AP(
                            tensor=u4.tensor,
                            offset=u4.offset + g * NCAP + nh * 2048,
                            ap=[list(u4.ap[0]), [4 * NCAP, 8], [1, NCAP]])
                        nc.tensor.matmul(
                            s0_ps[:, nh * 512:(nh + 1) * 512],
                            sel_sb[:, g * NCAP:(g + 1) * NCAP],
                            rhs,
                            start=(t == 0), stop=(t == NT - 1),
                            skip_group_check=True)
                if blk >= RESB:
                    nc.scalar.dma_start(out=u_spill_d[blk - RESB], in_=u4)

            prefetch_spills(2)
            allreduce_post(s0_ps, 0)
            s_glob = allreduce_fetch(0)
            v_sb = squash(s_glob, 1.0 / NCAP, bf16=True)
            vb = broadcast_v(v_sb, 0)

            # ================= Routing iterations 1 and 2 ===================
            for it in (1, 2):
                s_ps = psa.tile([B, ND], FP, tag="s_acc")

                def stage_a(blk, it=it):
                    """load + agreement product + tree reduce -> b logits."""
                    if blk < RESB:
                        u4 = u_res[:, blk * GND:(blk + 1) * GND]
                    elif blk in pref:
                        u4 = pref.pop(blk)
                    else:
                        u4 = ulp.tile([128, GND], BF, tag="u4", name="u4")
                        nc.sync.dma_start(out=u4, in_=u_spill_d[blk - RESB])
                    tmp4 = tp.tile([128, GND], BF, tag="tmp4", name="tmp4")
                    nc.vector.tensor_mul(tmp4, u4, vb)
                    # contiguous tree reduction over d: cols are (d, g, n)
                    b_slice = b_all[:, blk * 4 * NCAP:(blk + 1) * 4 * NCAP]
                    with nc.allow_low_precision(
                            reason="bf16 routing logits, validated offline"):
                        nc.vector.tensor_add(
                            tmp4[:, 0:2048], tmp4[:, 0:2048], tmp4[:, 2048:4096])
                        nc.vector.tensor_add(
                            tmp4[:, 0:1024], tmp4[:, 0:1024], tmp4[:, 1024:2048])
                        nc.vector.tensor_add(
                            tmp4[:, 0:512], tmp4[:, 0:512], tmp4[:, 512:1024])
                        if it == 1:
                            nc.vector.tensor_add(
                                b_slice, tmp4[:, 0:256], tmp4[:, 256:512])
                        else:
                            agr4 = sp.tile([128, 4 * NCAP], BF, tag="agr4",
                                           name="agr4")
                            nc.vector.tensor_add(
                                agr4, tmp4[:, 0:256], tmp4[:, 256:512])
                            nc.vector.tensor_add(b_slice, b_slice, agr4)
                    return u4

                def stage_b(blk):
                    """softmax over n -> c_rep [128, (d,g,n)] bf16."""
                    b_slice = b_all[:, blk * 4 * NCAP:(blk + 1) * 4 * NCAP]
                    c_un = sp.tile([128, 4 * NCAP], BF, tag="c_un",
                                   name="c_un")
                    nc.scalar.activation(out=c_un, in_=b_slice,
                                         func=mybir.ActivationFunctionType.Exp)
                    se4 = sp.tile([128, 4], FP, tag="se4", name="se4")
                    nc.vector.reduce_sum(
                        out=se4,
                        in_=c_un.rearrange("p (t n) -> p t n", n=NCAP),
                        axis=mybir.AxisListType.X)
                    rec4 = sp.tile([128, 4], FP, tag="rec4", name="rec4")
                    nc.vector.reciprocal(out=rec4, in_=se4)
                    c_bf = sp.tile([128, 4 * NCAP], BF, tag="c_bf",
                                   name="c_bf")
                    rec_b = bass.AP(tensor=rec4.tensor, offset=rec4.offset,
                                    ap=[list(rec4.ap[0]), [1, 4], [0, NCAP]])
                    with nc.allow_low_precision(reason="bf16 routing weights"):
                        nc.vector.tensor_mul(
                            c_bf.rearrange("p (t n) -> p t n", n=NCAP),
                            c_un.rearrange("p (t n) -> p t n", n=NCAP),
                            rec_b)
                    # replicate c over d on the scalar engine: (g,n) -> (d,g,n)
                    c_rep = crp.tile([128, GND], BF, tag="c_rep",
                                     name="c_rep")
                    c_src = bass.AP(tensor=c_bf.tensor, offset=c_bf.offset,
                                    ap=[list(c_bf.ap[0]), [0, D],
                                        [1, 4 * NCAP]])
                    nc.scalar.copy(
                        out=c_rep.rearrange("p (d t) -> p d t", d=D),
                        in_=c_src)
                    return c_rep

                def stage_c(blk, u4, c_rep):
                    """cu product + selector matmuls for one block."""
                    cu4 = cup.tile([128, GND], BF, tag="cu4", name="cu4")
                    nc.vector.tensor_mul(cu4, u4, c_rep)
                    t0 = blk * 4
                    for g in range(4):
                        for nh in range(2):
                            rhs = bass.AP(
                                tensor=cu4.tensor,
                                offset=cu4.offset + g * NCAP + nh * 2048,
                                ap=[list(cu4.ap[0]), [4 * NCAP, 8], [1, NCAP]])
                            nc.tensor.matmul(
                                s_ps[:, nh * 512:(nh + 1) * 512],
                                sel_sb[:, g * NCAP:(g + 1) * NCAP],
                                rhs,
                                start=(t0 + g == 0), stop=(t0 + g == NT - 1),
                                skip_group_check=True)

                # software pipeline: blk's softmax overlaps with (blk-1)'s
                # cu product so DVE never waits on the ACT exp.
                pending = None
                for blk in range(NBLK):
                    u4 = stage_a(blk)
                    if pending is not None:
                        stage_c(pending[0], pending[1], pending[2])
                    c_rep = stage_b(blk)
                    pending = (blk, u4, c_rep)
                stage_c(pending[0], pending[1], pending[2])
                if it == 1:
                    prefetch_spills(2)
                allreduce_post(s_ps, it)
                s_glob = allreduce_fetch(it)
                v_sb = squash(s_glob, 1.0, bf16=(it < 2))
                if it < 2:
                    vb = broadcast_v(v_sb, 1)
                else:
                    nc.sync.dma_start(out=v_out[:, :], in_=v_sb)

    nc.compile()
    return nc


_CACHED = {}


def _get_program():
    if "nc" not in _CACHED:
        _CACHED["nc"] = _build_program()
    return _CACHED["nc"]


def kernel(x, W, bias):
    x = np.asarray(x, dtype=np.float32)
    W = np.asarray(W, dtype=np.float32)
    bias = np.asarray(bias, dtype=np.float32)

    wxbd_all, xbd_all, sel, bias_dn = _host_prep(x, W, bias)
    nc = _get_program()

    in_maps = []
    for c in range(CORES):
        in_maps.append({
            "wxbd": wxbd_all[c],
            "xbd": xbd_all[c],
            "sel": sel,
            "bias_flat": bias_dn,
        })
    res = run_bass_kernel_spmd(nc, in_maps, core_ids=list(range(CORES)))
    _CACHED["last_results"] = res
    # v_out columns are (d, n): reshape to [B, n, d]
    v = res.results[0]["v_out"].reshape(B, D, NCAP).transpose(0, 2, 1)
    return np.ascontiguousarray(v).astype(np.float32)


# revision 11
# speedup vs baseline: 1.0857x; 1.0857x over previous
"""ClassCapsule dynamic-routing kernel for 8 Trainium2 NeuronCores.

Problem (hardcoded shapes):
    x:    [64, 2048, 16]  fp32
    W:    [2048, 16, 1024] fp32
    bias: [64, 16]        fp32
    out:  [64, 64, 16]    fp32  (squeezed v after 3 routing iterations)

Strategy (in_caps-sharded, s-AllReduce per iteration):
  - IC=2048 in_caps split across 8 cores (256 each); every core holds all
    64 batches.  W traffic per core is 1/8th of the replicated layout.
  - u_hat tiles are [128=(s8,b16), 4096=(d16,g4,n64)] bf16 per 8-in_cap
    block: the (d, g, n) column order makes every d-tree-reduce level a
    contiguous slice.
  - Projection runs in two passes: pass 1 streams W and accumulates the
    iteration-0 s (c0 uniform) as a dense GEMM over K=(in_cap,e), so its
    AllReduce is posted early and hides under pass 2 (the u_hat matmuls
    with a full-K block-diagonal x operand + PSUM->SBUF copies + spill).
  - u_hat kept SBUF-resident for the first RESB block groups, spilled to
    DRAM per block group [128,4096] and re-read in iters 1-2 (prefetched
    across the AllReduce gap; spill-out uses the vector-engine DMA queue
    so it never serializes with the sync-queue loads).
  - Per iteration per block: agreement product + contiguous 4-level tree
    reduce (DVE), softmax over n (ACT exp, DVE smalls), c replicated over
    d by a broadcast copy on the scalar engine, cu product (DVE), s
    accumulated with selector matmuls (PE), then an 8-core fp32 AllReduce
    of the partial s [64,1024] and squash.  GPSIMD is intentionally idle:
    its SBUF port pair is an exclusive lock shared with the DVE, so any
    GPSIMD tensor work just serializes against the DVE.
"""

import numpy as np
import ml_dtypes

import concourse.bass as bass
import concourse.tile as tile
from concourse import bacc, mybir
from concourse.bass_utils import run_bass_kernel_spmd

# ---------------------------------------------------------------- constants
B, IC, E = 64, 2048, 16          # batch, in_caps, in_dim
NCAP, D = 64, 16                 # n_caps, cap_dim
ND = NCAP * D                    # 1024
CORES = 8
ICL = IC // CORES                # 256 local in_caps
NBLK = ICL // 8                  # 32 blocks of 8 in_caps
NT = NBLK * 4                    # 128 u_hat tiles [(s,b16), 1024]
RESB = 9                        # block groups resident in SBUF
GND = 4 * ND                     # group width (4096)
EPS = 1e-7

FP = mybir.dt.float32
BF = mybir.dt.bfloat16
BF_NP = ml_dtypes.bfloat16


def _host_prep(x, W, bias):
    """Build per-core host-side tensors (all bf16 except bias)."""
    wxbd_all = []
    xbd_all = []
    for c in range(CORES):
        i0 = c * ICL
        w8 = W[i0:i0 + ICL].reshape(NBLK, 128, ND)          # [(blk),(s,e),nd]
        # reorder W columns (n,d) -> (d,n)
        w8 = w8.reshape(NBLK, 128, NCAP, D).transpose(0, 1, 3, 2)
        w8 = np.ascontiguousarray(w8).reshape(NBLK, 128, ND)
        xc = x[:, i0:i0 + ICL, :]                           # [64, 256, 16]
        # xr[blk, s, e, bg, b] = x[bg*16+b, i0+blk*8+s, e]
        xr = xc.transpose(1, 2, 0).reshape(NBLK, 8, E, 4, 16)
        wxbd_all.append(w8.astype(BF_NP))
        # block-diagonal x: xbd[blk, bg, s*16+e, s*16+b] = xr[blk, s, e, bg, b]
        xbd = np.zeros((NBLK, 4, 128, 128), np.float32)
        for s in range(8):
            xbd[:, :, s * 16:(s + 1) * 16, s * 16:(s + 1) * 16] = \
                xr[:, s].transpose(0, 2, 1, 3)
        xbd_all.append(xbd.astype(BF_NP))

    # selector: sel[p, g*64 + m] = 1 if m == g*16 + p%16
    sel = np.zeros((128, 4 * NCAP), np.float32)
    p = np.arange(128)
    for g in range(4):
        sel[p, g * NCAP + g * 16 + (p % 16)] = 1.0
    sel = sel.astype(BF_NP)

    # bias in (d, n) order
    bias_dn = np.ascontiguousarray(bias.T.reshape(1, ND)).astype(np.float32)
    return wxbd_all, xbd_all, sel, bias_dn


def _build_program():
    nc = bacc.Bacc("TRN2", target_bir_lowering=False, num_devices=CORES)

    wxbd_d = nc.dram_tensor("wxbd", [NBLK, 128, ND], BF,
                            kind="ExternalInput")
    xbd_d = nc.dram_tensor("xbd", [NBLK, 4, 128, 128], BF,
                           kind="ExternalInput")
    sel_d = nc.dram_tensor("sel", [128, 4 * NCAP], BF, kind="ExternalInput")
    bias_d = nc.dram_tensor("bias_flat", [1, ND], FP, kind="ExternalInput")
    v_out = nc.dram_tensor("v_out", [B, ND], FP, kind="ExternalOutput")

    u_spill_d = nc.dram_tensor("u_spill", [NBLK - RESB, 128, GND], BF)
    v_scr = [nc.dram_tensor(f"v_scr{r}", [B, ND], BF) for r in range(2)]
    ar_in = [nc.dram_tensor(f"ar_in{r}", [B, ND], FP) for r in range(3)]
    ar_out = [nc.dram_tensor(f"ar_out{r}", [B, ND], FP, addr_space="Shared")
              for r in range(3)]

    with tile.TileContext(nc) as tc:
        with (
            tc.tile_pool(name="wp", bufs=4) as wp,
            tc.tile_pool(name="xp", bufs=4) as xp,         # xbd stream
            tc.tile_pool(name="ul", bufs=3) as ulp,        # spill load/stage
            tc.tile_pool(name="tp", bufs=1) as tp,         # u*v products
            tc.tile_pool(name="crp", bufs=2) as crp,       # c replicated
            tc.tile_pool(name="cup", bufs=2) as cup,       # u*c products
            tc.tile_pool(name="smalls", bufs=2) as sp,
            tc.tile_pool(name="sq", bufs=1) as qp,         # squash [64,*]
            tc.tile_pool(name="consts", bufs=1) as cp,
            tc.tile_pool(name="vbp", bufs=1) as vbp,
            tc.tile_pool(name="ures", bufs=1) as urp,
            tc.tile_pool(name="bstate", bufs=1) as bsp,
            tc.tile_pool(name="ps", bufs=3, space="PSUM") as psp,
            tc.tile_pool(name="ps_acc", bufs=1, space="PSUM") as psa,
        ):
            # ---- constants resident in SBUF
            sel_sb = cp.tile([128, 4 * NCAP], BF)
            nc.sync.dma_start(out=sel_sb, in_=sel_d[:, :])
            bias_sb = cp.tile([B, ND], FP)
            bias_src = bass.AP(tensor=bias_d, offset=0, ap=[[0, B], [1, ND]])
            nc.sync.dma_start(out=bias_sb, in_=bias_src)
            eps_t = cp.tile([B, 1], FP)
            nc.vector.memset(eps_t, EPS)

            # persistent state
            u_res = urp.tile([128, RESB * GND], BF)        # resident u_hat
            b_all = bsp.tile([128, NT * NCAP], BF)         # routing logits

            # ---------------- squash helper: v = squash(s*scale + bias)
            def squash(s_in, scale, bf16=False):
                s_sb = qp.tile([B, ND], FP, tag="s_sb")
                nc.vector.scalar_tensor_tensor(
                    out=s_sb, in0=s_in, scalar=float(scale), in1=bias_sb,
                    op0=mybir.AluOpType.mult, op1=mybir.AluOpType.add)
                sq = qp.tile([B, ND], FP, tag="sq")
                nc.vector.tensor_mul(sq, s_sb, s_sb)
                # contiguous tree reduce over d (columns are (d, n))
                nc.vector.tensor_add(sq[:, 0:512], sq[:, 0:512],
                                     sq[:, 512:1024])
                nc.vector.tensor_add(sq[:, 0:256], sq[:, 0:256], sq[:, 256:512])
                nc.vector.tensor_add(sq[:, 0:128], sq[:, 0:128], sq[:, 128:256])
                nsq = sp.tile([B, NCAP], FP, tag="nsq")
                nc.vector.tensor_add(nsq, sq[:, 0:64], sq[:, 64:128])
                norm = sp.tile([B, NCAP], FP, tag="norm")
                nc.scalar.activation(out=norm, in_=nsq,
                                     func=mybir.ActivationFunctionType.Sqrt,
                                     bias=eps_t[:, :], scale=1.0)
                den = sp.tile([B, NCAP], FP, tag="den")
                nc.vector.scalar_tensor_tensor(
                    out=den, in0=nsq, scalar=float(EPS + 1.0), in1=norm,
                    op0=mybir.AluOpType.add, op1=mybir.AluOpType.mult)
                rden = sp.tile([B, NCAP], FP, tag="rden")
                nc.vector.reciprocal(out=rden, in_=den)
                fac = sp.tile([B, NCAP], FP, tag="fac")
                nc.vector.scalar_tensor_tensor(
                    out=fac, in0=nsq, scalar=float(EPS), in1=rden,
                    op0=mybir.AluOpType.add, op1=mybir.AluOpType.mult)
                if bf16:
                    out_t = qp.tile([B, ND], BF, tag="v_bf16", name="v_bf16")
                else:
                    out_t = qp.tile([B, ND], FP, tag="v_sb", name="v_sb")
                fac_b = bass.AP(tensor=fac.tensor, offset=fac.offset,
                                ap=[list(fac.ap[0]), [0, D], [1, NCAP]])
                with nc.allow_low_precision(reason="bf16 v for broadcast"):
                    nc.vector.tensor_mul(
                        out_t.rearrange("p (d n) -> p d n", n=NCAP),
                        s_sb.rearrange("p (d n) -> p d n", n=NCAP),
                        fac_b)
                return out_t

            def allreduce_post(s_ps, r):
                """partial s (PSUM): copy out and launch the AllReduce."""
                s_loc = qp.tile([B, ND], FP, tag="s_loc")
                nc.scalar.copy(out=s_loc, in_=s_ps)
                nc.scalar.dma_start(out=ar_in[r][:, :], in_=s_loc)
                nc.gpsimd.collective_compute(
                    "AllReduce",
                    mybir.AluOpType.add,
                    replica_groups=[list(range(CORES))],
                    ins=[ar_in[r][:, :].opt()],
                    outs=[ar_out[r][:, :].opt()],
                )

            def allreduce_fetch(r):
                s_glob = qp.tile([B, ND], FP, tag="s_glob")
                nc.scalar.dma_start(out=s_glob, in_=ar_out[r][:, :])
                return s_glob

            def broadcast_v(v_bf, r):
                """v [64,(d,n)] bf16 -> vb [128, (d,g,n)] bf16."""
                nc.scalar.dma_start(out=v_scr[r][:, :], in_=v_bf)
                vb = vbp.tile([128, GND], BF, tag="vb", name="vb")
                for g in range(4):
                    # dest cols (d, n) for fixed g; src v_scr[b(g,b16), (d,n)]
                    dst = bass.AP(tensor=vb.tensor,
                                  offset=vb.offset + g * NCAP,
                                  ap=[list(vb.ap[0]), [4 * NCAP, D], [1, NCAP]])
                    src = bass.AP(tensor=v_scr[r], offset=g * 16 * ND,
                                  ap=[[0, 8], [ND, 16], [NCAP, D], [1, NCAP]])
                    nc.scalar.dma_start(out=dst, in_=src)
                return vb

            # spill prefetch bookkeeping
            pref = {}

            def prefetch_spills(n):
                for blk in range(RESB, min(RESB + n, NBLK)):
                    t = ulp.tile([128, GND], BF, tag="u4", name="u4")
                    nc.sync.dma_start(out=t, in_=u_spill_d[blk - RESB])
                    pref[blk] = t

            # ========== Phase P: u_hat projection + iter-0 s + spill =======
            # The iter-0 selector matmuls read u4 (post-copy), so they are
            # issued one block late to keep the in-order PE queue from
            # stalling on the ACT/DVE copies.
            s0_ps = psa.tile([B, ND], FP, tag="s_acc")

            def s0_sel_mms(blk, u4):
                for g in range(4):
                    t = blk * 4 + g
                    for nh in range(2):
                        rhs = bass.AP(
                            tensor=u4.tensor,
                            offset=u4.offset + g * NCAP + nh * 2048,
                            ap=[list(u4.ap[0]), [4 * NCAP, 8], [1, NCAP]])
                        nc.tensor.matmul(
                            s0_ps[:, nh * 512:(nh + 1) * 512],
                            sel_sb[:, g * NCAP:(g + 1) * NCAP],
                            rhs,
                            start=(t == 0), stop=(t == NT - 1),
                            skip_group_check=True)

            psel = None
            for blk in range(NBLK):
                wt = wp.tile([128, ND], BF, tag="w")
                nc.sync.dma_start(out=wt, in_=wxbd_d[blk])
                xt = xp.tile([128, 512], BF, tag="x")
                xsrc = bass.AP(tensor=xbd_d, offset=blk * 4 * 128 * 128,
                               ap=[[128, 128], [128 * 128, 4], [1, 128]])
                nc.sync.dma_start(out=xt, in_=xsrc)
                if blk < RESB:
                    u4 = u_res[:, blk * GND:(blk + 1) * GND]
                else:
                    u4 = ulp.tile([128, GND], BF, tag="u4")
                for g in range(4):
                    u_ps = psp.tile([128, ND], FP, tag="u_ps")
                    for nh in range(2):
                        nc.tensor.matmul(
                            u_ps[:, nh * 512:(nh + 1) * 512],
                            xt[:, g * 128:(g + 1) * 128],
                            wt[:, nh * 512:(nh + 1) * 512],
                            start=True, stop=True,
                            skip_group_check=True)
                    if g == 0 and psel is not None:
                        s0_sel_mms(psel[0], psel[1])
                    # copy u_ps [128,(d,n)] into u4 cols (d,g,n) at slice g
                    u_dst = bass.AP(tensor=u4.tensor,
                                    offset=u4.offset + g * NCAP,
                                    ap=[list(u4.ap[0]), [4 * NCAP, D],
                                        [1, NCAP]])
                    u_src = u_ps.rearrange("p (d n) -> p d n", n=NCAP)
                    if g % 2 == 0:
                        nc.scalar.copy(out=u_dst, in_=u_src)
                    else:
                        nc.vector.tensor_copy(out=u_dst, in_=u_src)
                if blk >= RESB:
                    nc.scalar.dma_start(out=u_spill_d[blk - RESB], in_=u4)
                psel = (blk, u4)
            s0_sel_mms(psel[0], psel[1])

            prefetch_spills(2)
            allreduce_post(s0_ps, 0)
            s_glob = allreduce_fetch(0)
            v_sb = squash(s_glob, 1.0 / NCAP, bf16=True)
            vb = broadcast_v(v_sb, 0)

            # ================= Routing iterations 1 and 2 ===================
            for it in (1, 2):
                s_ps = psa.tile([B, ND], FP, tag="s_acc")

                def stage_a(blk, it=it):
                    """load + agreement product + tree reduce -> b logits."""
                    if blk < RESB:
                        u4 = u_res[:, blk * GND:(blk + 1) * GND]
                    elif blk in pref:
                        u4 = pref.pop(blk)
                    else:
                        u4 = ulp.tile([128, GND], BF, tag="u4", name="u4")
                        nc.sync.dma_start(out=u4, in_=u_spill_d[blk - RESB])
                    tmp4 = tp.tile([128, GND], BF, tag="tmp4", name="tmp4")
                    nc.vector.tensor_mul(tmp4, u4, vb)
                    # contiguous tree reduction over d: cols are (d, g, n)
                    b_slice = b_all[:, blk * 4 * NCAP:(blk + 1) * 4 * NCAP]
                    with nc.allow_low_precision(
                            reason="bf16 routing logits, validated offline"):
                        nc.vector.tensor_add(
                            tmp4[:, 0:2048], tmp4[:, 0:2048], tmp4[:, 2048:4096])
                        nc.vector.tensor_add(
                            tmp4[:, 0:1024], tmp4[:, 0:1024], tmp4[:, 1024:2048])
                        nc.vector.tensor_add(
                            tmp4[:, 0:512], tmp4[:, 0:512], tmp4[:, 512:1024])
                        if it == 1:
                            nc.vector.tensor_add(
                                b_slice, tmp4[:, 0:256], tmp4[:, 256:512])
                        else:
                            agr4 = sp.tile([128, 4 * NCAP], BF, tag="agr4",
                                           name="agr4")
                            nc.vector.tensor_add(
                                agr4, tmp4[:, 0:256], tmp4[:, 256:512])
                            nc.vector.tensor_add(b_slice, b_slice, agr4)
                    return u4

                def stage_b(blk):
                    """softmax over n -> c_rep [128, (d,g,n)] bf16."""
                    b_slice = b_all[:, blk * 4 * NCAP:(blk + 1) * 4 * NCAP]
                    c_un = sp.tile([128, 4 * NCAP], BF, tag="c_un",
                                   name="c_un")
                    nc.scalar.activation(out=c_un, in_=b_slice,
                                         func=mybir.ActivationFunctionType.Exp)
                    se4 = sp.tile([128, 4], FP, tag="se4", name="se4")
                    nc.vector.reduce_sum(
                        out=se4,
                        in_=c_un.rearrange("p (t n) -> p t n", n=NCAP),
                        axis=mybir.AxisListType.X)
                    rec4 = sp.tile([128, 4], FP, tag="rec4", name="rec4")
                    nc.vector.reciprocal(out=rec4, in_=se4)
                    c_bf = sp.tile([128, 4 * NCAP], BF, tag="c_bf",
                                   name="c_bf")
                    rec_b = bass.AP(tensor=rec4.tensor, offset=rec4.offset,
                                    ap=[list(rec4.ap[0]), [1, 4], [0, NCAP]])
                    with nc.allow_low_precision(reason="bf16 routing weights"):
                        nc.vector.tensor_mul(
                            c_bf.rearrange("p (t n) -> p t n", n=NCAP),
                            c_un.rearrange("p (t n) -> p t n", n=NCAP),
                            rec_b)
                    # replicate c over d on the scalar engine: (g,n) -> (d,g,n)
                    c_rep = crp.tile([128, GND], BF, tag="c_rep",
                                     name="c_rep")
                    c_src = bass.AP(tensor=c_bf.tensor, offset=c_bf.offset,
                                    ap=[list(c_bf.ap[0]), [0, D],
                                        [1, 4 * NCAP]])
                    nc.scalar.copy(
                        out=c_rep.rearrange("p (d t) -> p d t", d=D),
                        in_=c_src)
                    return c_rep

                def stage_c(blk, u4, c_rep):
                    """cu product + selector matmuls for one block."""
                    cu4 = cup.tile([128, GND], BF, tag="cu4", name="cu4")
                    nc.vector.tensor_mul(cu4, u4, c_rep)
                    t0 = blk * 4
                    for g in range(4):
                        for nh in range(2):
                            rhs = bass.AP(
                                tensor=cu4.tensor,
                                offset=cu4.offset + g * NCAP + nh * 2048,
                                ap=[list(cu4.ap[0]), [4 * NCAP, 8], [1, NCAP]])
                            nc.tensor.matmul(
                                s_ps[:, nh * 512:(nh + 1) * 512],
                                sel_sb[:, g * NCAP:(g + 1) * NCAP],
                                rhs,
                                start=(t0 + g == 0), stop=(t0 + g == NT - 1),
                                skip_group_check=True)

                # software pipeline: blk's softmax overlaps with (blk-1)'s
                # cu product so DVE never waits on the ACT exp.
                pending = None
                for blk in range(NBLK):
                    u4 = stage_a(blk)
                    if pending is not None:
                        stage_c(pending[0], pending[1], pending[2])
                    c_rep = stage_b(blk)
                    pending = (blk, u4, c_rep)
                stage_c(pending[0], pending[1], pending[2])
                if it == 1:
                    prefetch_spills(2)
                allreduce_post(s_ps, it)
                s_glob = allreduce_fetch(it)
                v_sb = squash(s_glob, 1.0, bf16=(it < 2))
                if it < 2:
                    vb = broadcast_v(v_sb, 1)
                else:
                    nc.sync.dma_start(out=v_out[:, :], in_=v_sb)

    nc.compile()
    return nc


_CACHED = {}


def _get_program():
    if "nc" not in _CACHED:
        _CACHED["nc"] = _build_program()
    return _CACHED["nc"]


def kernel(x, W, bias):
    x = np.asarray(x, dtype=np.float32)
    W = np.asarray(W, dtype=np.float32)
    bias = np.asarray(bias, dtype=np.float32)

    wxbd_all, xbd_all, sel, bias_dn = _host_prep(x, W, bias)
    nc = _get_program()

    in_maps = []
    for c in range(CORES):
        in_maps.append({
            "wxbd": wxbd_all[c],
            "xbd": xbd_all[c],
            "sel": sel,
            "bias_flat": bias_dn,
        })
    res = run_bass_kernel_spmd(nc, in_maps, core_ids=list(range(CORES)))
    _CACHED["last_results"] = res
    # v_out columns are (d, n): reshape to [B, n, d]
    v = res.results[0]["v_out"].reshape(B, D, NCAP).transpose(0, 2, 1)
    return np.ascontiguousarray(v).astype(np.float32)
